# revision 5
# baseline (speedup 1.0000x reference)
"""2-layer LSTM decoder on trn2 — 8-way tensor-parallel, hybrid fp8/bf16.

Each core owns a 128-unit slice of H for each gate (gate-on-partition psum,
batch 512 on the free dim). Instruction count is the cost driver here, so:
  * W_hh0 and W_ih1 are fp8e4m3 with DoubleRow matmuls (K=256 per call),
    halving their call count; the recurrent h0 is stored/gathered as fp8.
  * W_hh1 stays bf16 (its quantization alone costs ~1.7e-2 output error),
    as do the x chunk and the projection.
  * All stationaries are pre-scaled x1024 (fp8 subnormal avoidance); the
    descale rides the activation `scale` operand for free.
  * L0's biases are folded into the matmul via a ones-row in x, so one
    fused Sigmoid covers the i/f/o banks; gates are bank-ordered [i,f,o,g].
  * One AllGather per layer per step (fp8 for h0, bf16 for h1); the gather
    unpack is a single rearranged-AP DMA.
Gate psum is one 4-bank tile per layer; pred rides the same psum ring.
"""
import numpy as np
import ml_dtypes


import concourse.bass as bass
import concourse.mybir as mybir
import concourse.tile as tile
from concourse import bacc

F32 = mybir.dt.float32
BF16 = mybir.dt.bfloat16
FP8 = mybir.dt.float8e4
AF = mybir.ActivationFunctionType
ALU = mybir.AluOpType
PM = mybir.MatmulPerfMode

B, T_FULL, F, H, GE = 512, 168, 32, 1024, 16
N_CORES = 8
NKH = 8          # 128-row chunks per 1024 contraction
NDR = 4          # 256-row DoubleRow chunks per 1024 contraction
GP = [0, 1, 3, 2]   # psum bank b holds torch gate GP[b]:  [i, f, o, g]
WS = 1024.0      # stationary pre-scale
E4 = ml_dtypes.float8_e4m3fn
BF = ml_dtypes.bfloat16


def prep_host(inputs, T):
    inp = {k: np.asarray(v) for k, v in inputs.items()}
    gv_all = inp["group_emb"][inp["group_ids"].astype(np.int64)]  # (B, GE)
    b0 = (inp["b_ih0"] + inp["b_hh0"]).astype(np.float32)
    b1 = (inp["b_ih1"] + inp["b_hh1"]).astype(np.float32)

    W_ih0 = inp["W_ih0"].astype(np.float32)   # (4096, 49)
    W_hh0 = inp["W_hh0"].astype(np.float32)   # (4096, 1024)
    W_ih1 = inp["W_ih1"].astype(np.float32)
    W_hh1 = inp["W_hh1"].astype(np.float32)
    wp = inp["W_proj"].astype(np.float32)[0]  # (1024,)

    shared = dict(
        knT=np.ascontiguousarray(
            inp["dec_known"][:, :T, :].transpose(1, 2, 0)).astype(BF),  # (T,32,B)
        yT=np.ascontiguousarray(inp["target_y"][:, :T, 0].T).astype(BF),
        gvT=np.ascontiguousarray(gv_all.T).astype(BF),
        leT=np.ascontiguousarray(inp["last_enc_consumption"].T).astype(BF),
        h0i8=np.ascontiguousarray(
            inp["h0"][0].T.reshape(NKH, 128, B).transpose(1, 0, 2)).astype(E4),
        h1i=np.ascontiguousarray(
            inp["h0"][1].T.reshape(NKH, 128, B).transpose(1, 0, 2)).astype(BF),
        ones=np.ones((1, B), BF),
    )

    per_core = []
    for c in range(N_CORES):
        cols = [1024 * GP[b] + 128 * c for b in range(4)]  # gate-unit col base

        # x-chunk stationary (50, 4*128) bf16: rows 0-48 = W_ih0 cols, row 49
        # = bias row b0; all x1024.
        w0x = np.zeros((50, 4, 128), np.float32)
        for b in range(4):
            w0x[0:49, b, :] = W_ih0.T[:, cols[b]:cols[b] + 128]
            w0x[49, b, :] = b0[cols[b]:cols[b] + 128]
        w0x = np.ascontiguousarray((w0x * WS).reshape(50, 512)).astype(BF)

        def dr_tiles(Wt):  # Wt: (1024, 4096) = W.T ; -> (128, NDR,4,2,128) fp8
            a = np.zeros((128, NDR, 4, 2, 128), np.float32)
            for c4 in range(NDR):
                for b in range(4):
                    for i in range(2):
                        rows = slice(256 * c4 + 128 * i, 256 * c4 + 128 * (i + 1))
                        a[:, c4, b, i, :] = Wt[rows, cols[b]:cols[b] + 128]
            return np.ascontiguousarray(
                (a * WS).reshape(128, NDR * 1024)).astype(E4)

        whh0 = dr_tiles(W_hh0.T)
        wih1 = dr_tiles(W_ih1.T)

        whh1 = np.zeros((128, NKH, 4, 128), np.float32)
        for k in range(NKH):
            for b in range(4):
                whh1[:, k, b, :] = W_hh1.T[128 * k:128 * (k + 1),
                                           cols[b]:cols[b] + 128]
        whh1 = np.ascontiguousarray((whh1 * WS).reshape(128, 4096)).astype(BF)

        b1c = np.zeros((128, 4), np.float32)
        for b in range(4):
            b1c[:, b] = b1[cols[b]:cols[b] + 128]

        d = dict(
            w0x=w0x, whh0=whh0, wih1=wih1, whh1=whh1, b1c=b1c,
            wpc=np.ascontiguousarray(wp[128 * c:128 * (c + 1), None]).astype(BF),
            c0i=np.ascontiguousarray(
                inp["c0"][0, :, 128 * c:128 * (c + 1)].T).astype(np.float32),
            c1i=np.ascontiguousarray(
                inp["c0"][1, :, 128 * c:128 * (c + 1)].T).astype(np.float32),
        )
        per_core.append(d)
    tf_mask = [int(v) for v in np.asarray(inp["tf_mask"]).reshape(-1)][:T]
    b_proj = float(np.asarray(inp["b_proj"]).reshape(-1)[0])
    return shared, per_core, tf_mask, b_proj


def build_module(T, tf_mask, b_proj, rep=1):
    nc = bacc.Bacc(target_bir_lowering=False)

    w0x_d = nc.dram_tensor("w0x", [50, 512], BF16, kind="ExternalInput")
    whh0_d = nc.dram_tensor("whh0", [128, NDR * 1024], FP8, kind="ExternalInput")
    wih1_d = nc.dram_tensor("wih1", [128, NDR * 1024], FP8, kind="ExternalInput")
    whh1_d = nc.dram_tensor("whh1", [128, 4096], BF16, kind="ExternalInput")
    b1_d = nc.dram_tensor("b1c", [128, 4], F32, kind="ExternalInput")
    wpc_d = nc.dram_tensor("wpc", [128, 1], BF16, kind="ExternalInput")
    knT_d = nc.dram_tensor("knT", [T, F, B], BF16, kind="ExternalInput")
    yT_d = nc.dram_tensor("yT", [T, B], BF16, kind="ExternalInput")
    gvT_d = nc.dram_tensor("gvT", [GE, B], BF16, kind="ExternalInput")
    leT_d = nc.dram_tensor("leT", [1, B], BF16, kind="ExternalInput")
    ones_d = nc.dram_tensor("ones", [1, B], BF16, kind="ExternalInput")
    h0i8_d = nc.dram_tensor("h0i8", [128, NKH, B], FP8, kind="ExternalInput")
    h1i_d = nc.dram_tensor("h1i", [128, NKH, B], BF16, kind="ExternalInput")
    c0i_d = nc.dram_tensor("c0i", [128, B], F32, kind="ExternalInput")
    c1i_d = nc.dram_tensor("c1i", [128, B], F32, kind="ExternalInput")
    out_d = nc.dram_tensor("out", [T, B], F32, kind="ExternalOutput")

    RG = [list(range(N_CORES))]
    DS = 1.0 / WS

    with tile.TileContext(nc) as tc:
        with tc.tile_pool(name="const", bufs=1) as const, \
             tc.tile_pool(name="hfp", bufs=2) as hfp, \
             tc.tile_pool(name="act", bufs=4) as actp, \
             tc.tile_pool(name="st", bufs=2) as stp, \
             tc.tile_pool(name="sm", bufs=2) as smp, \
             tc.tile_pool(name="gps", bufs=2, space="PSUM") as gpsum, \
             tc.tile_pool(name="dram", bufs=2, space="DRAM") as dramp:

            w0x_sb = const.tile([50, 512], BF16)
            nc.sync.dma_start(out=w0x_sb[:], in_=w0x_d[:])
            whh0_sb = const.tile([128, NDR, 4, 2, 128], FP8)
            nc.sync.dma_start(out=whh0_sb[:],
                              in_=whh0_d[:].rearrange("p (c b i m) -> p c b i m",
                                                      c=NDR, b=4, i=2))
            wih1_sb = const.tile([128, NDR, 4, 2, 128], FP8)
            nc.sync.dma_start(out=wih1_sb[:],
                              in_=wih1_d[:].rearrange("p (c b i m) -> p c b i m",
                                                      c=NDR, b=4, i=2))
            whh1_sb = const.tile([128, NKH, 4, 128], BF16)
            nc.sync.dma_start(out=whh1_sb[:],
                              in_=whh1_d[:].rearrange("p (k b m) -> p k b m",
                                                      k=NKH, b=4))
            b1_sb = const.tile([128, 4], F32)
            nc.sync.dma_start(out=b1_sb[:], in_=b1_d[:])
            wpc_sb = const.tile([128, 1], BF16)
            nc.sync.dma_start(out=wpc_sb[:], in_=wpc_d[:])
            ones8 = const.tile([8, 1], BF16)
            nc.vector.memset(ones8[:], 1.0)

            xt = []
            for i in range(2):
                x = const.tile([50, B], BF16, name=f"x{i}")
                nc.vector.memset(x[:], 0.0)
                nc.sync.dma_start(out=x[49:50, :], in_=ones_d[:])
                nc.sync.dma_start(out=x[33:49, :], in_=gvT_d[:])
                xt.append(x)

            for r in range(rep):
                def nm(s, t):
                    return f"{s}_r{r}_t{t}"

                c0_cur = stp.tile([128, B], F32, tag="c0", name=nm("c0", -1))
                nc.sync.dma_start(out=c0_cur[:], in_=c0i_d[:])
                c1_cur = stp.tile([128, B], F32, tag="c1", name=nm("c1", -1))
                nc.sync.dma_start(out=c1_cur[:], in_=c1i_d[:])
                h0f = hfp.tile([128, NKH, B], FP8, tag="h0f", name=nm("h0f", -1))
                nc.sync.dma_start(out=h0f[:], in_=h0i8_d[:])
                h1f = hfp.tile([128, NKH, B], BF16, tag="h1f", name=nm("h1f", -1))
                nc.sync.dma_start(out=h1f[:], in_=h1i_d[:])

                def allgather(hsl, tag, t, dt):
                    cin = dramp.tile([128, B], dt, tag=f"ci_{tag}",
                                     name=nm(f"ci{tag}", t))
                    cout = dramp.tile([NKH * 128, B], dt, tag=f"co_{tag}",
                                      name=nm(f"co{tag}", t), addr_space="Shared")
                    nc.sync.dma_start(out=cin[:], in_=hsl[:])
                    nc.gpsimd.collective_compute(
                        "AllGather", ALU.bypass, ins=[cin[:]], outs=[cout[:]],
                        replica_groups=RG)
                    hf = hfp.tile([128, NKH, B], dt, tag=tag, name=nm(f"hf{tag}", t))
                    nc.sync.dma_start(
                        out=hf[:], in_=cout[:].rearrange("(k p) b -> p k b", k=NKH))
                    return hf

                for t in range(T):
                    # ---- L0: W_hh0 fp8-DR chunks, then the bf16 x chunk
                    pg0 = gpsum.tile([128, 4, B], F32, tag="g", name=nm("g0", t))
                    for c4 in range(NDR):
                        for b in range(4):
                            nc.tensor.matmul(pg0[:, b, :], whh0_sb[:, c4, b],
                                             h0f[:, 2 * c4:2 * c4 + 2, :],
                                             start=(c4 == 0), stop=False,
                                             perf_mode=PM.DoubleRow)
                    # x(t): row 0 = prev output
                    xc = xt[t % 2]
                    if t == 0:
                        nc.sync.dma_start(out=xc[0:1, :], in_=leT_d[:])
                        nc.sync.dma_start(out=xc[1:33, :], in_=knT_d[0])
                    for b in range(4):
                        nc.tensor.matmul(pg0[:, b, :],
                                         w0x_sb[:, 128 * b:128 * (b + 1)],
                                         xc[:], start=False, stop=(b == 3))

                    # ---- cell0 (biases already in psum; banks i,f,o,g)
                    sio = actp.tile([128, 3, B], BF16, tag="sio", name=nm("sio", t))
                    nc.scalar.activation(sio[:], pg0[:, 0:3, :], AF.Sigmoid,
                                         scale=DS)
                    tg = actp.tile([128, B], BF16, tag="tg", name=nm("tg0", t))
                    nc.scalar.activation(tg[:], pg0[:, 3, :], AF.Tanh, scale=DS)
                    tmpf = actp.tile([128, B], F32, tag="tmpf", name=nm("tf0", t))
                    nc.vector.tensor_tensor(out=tmpf[:], in0=sio[:, 1, :],
                                            in1=c0_cur[:], op=ALU.mult)
                    tmpb = actp.tile([128, B], BF16, tag="tmpb", name=nm("tb0", t))
                    nc.vector.tensor_tensor(out=tmpb[:], in0=sio[:, 0, :],
                                            in1=tg[:], op=ALU.mult)
                    c0_new = stp.tile([128, B], F32, tag="c0", name=nm("c0", t))
                    nc.vector.tensor_tensor(out=c0_new[:], in0=tmpf[:],
                                            in1=tmpb[:], op=ALU.add)
                    tc0 = actp.tile([128, B], BF16, tag="tc", name=nm("tc0", t))
                    nc.scalar.activation(tc0[:], c0_new[:], AF.Tanh)
                    h0sl = stp.tile([128, B], FP8, tag="h0sl", name=nm("h0sl", t))
                    nc.vector.tensor_tensor(out=h0sl[:], in0=sio[:, 2, :],
                                            in1=tc0[:], op=ALU.mult)
                    c0_cur = c0_new
                    h0f = allgather(h0sl, "h0f", t, FP8)

                    # ---- L1: W_hh1 bf16 first (h1 of t-1), then W_ih1 fp8-DR
                    pg1 = gpsum.tile([128, 4, B], F32, tag="g", name=nm("g1", t))
                    for k in range(NKH):
                        for b in range(4):
                            nc.tensor.matmul(pg1[:, b, :], whh1_sb[:, k, b],
                                             h1f[:, k, :],
                                             start=(k == 0), stop=False)
                    for c4 in range(NDR):
                        for b in range(4):
                            nc.tensor.matmul(pg1[:, b, :], wih1_sb[:, c4, b],
                                             h0f[:, 2 * c4:2 * c4 + 2, :],
                                             start=False, stop=(c4 == NDR - 1),
                                             perf_mode=PM.DoubleRow)

                    # ---- cell1 (per-gate bias via ACT bias operand)
                    si = actp.tile([128, B], BF16, tag="si", name=nm("si1", t))
                    nc.scalar.activation(si[:], pg1[:, 0, :], AF.Sigmoid,
                                         bias=b1_sb[:, 0:1], scale=DS)
                    sf = actp.tile([128, B], BF16, tag="sf", name=nm("sf1", t))
                    nc.scalar.activation(sf[:], pg1[:, 1, :], AF.Sigmoid,
                                         bias=b1_sb[:, 1:2], scale=DS)
                    so = actp.tile([128, B], BF16, tag="so", name=nm("so1", t))
                    nc.scalar.activation(so[:], pg1[:, 2, :], AF.Sigmoid,
                                         bias=b1_sb[:, 2:3], scale=DS)
                    tg1 = actp.tile([128, B], BF16, tag="tg", name=nm("tg1", t))
                    nc.scalar.activation(tg1[:], pg1[:, 3, :], AF.Tanh,
                                         bias=b1_sb[:, 3:4], scale=DS)
                    tmpf1 = actp.tile([128, B], F32, tag="tmpf", name=nm("tf1", t))
                    nc.vector.tensor_tensor(out=tmpf1[:], in0=sf[:],
                                            in1=c1_cur[:], op=ALU.mult)
                    tmpb1 = actp.tile([128, B], BF16, tag="tmpb", name=nm("tb1", t))
                    nc.vector.tensor_tensor(out=tmpb1[:], in0=si[:], in1=tg1[:],
                                            op=ALU.mult)
                    c1_new = stp.tile([128, B], F32, tag="c1", name=nm("c1", t))
                    nc.vector.tensor_tensor(out=c1_new[:], in0=tmpf1[:],
                                            in1=tmpb1[:], op=ALU.add)
                    tc1 = actp.tile([128, B], BF16, tag="tc", name=nm("tc1", t))
                    nc.scalar.activation(tc1[:], c1_new[:], AF.Tanh)
                    h1sl = stp.tile([128, B], BF16, tag="h1sl", name=nm("h1sl", t))
                    nc.vector.tensor_tensor(out=h1sl[:], in0=so[:], in1=tc1[:],
                                            op=ALU.mult)
                    c1_cur = c1_new

                    # ---- local pred partial, rides the h1 AllGather
                    pp0 = gpsum.tile([1, B], F32, tag="g", name=nm("pp0", t))
                    nc.tensor.matmul(pp0[:], wpc_sb[:], h1sl[:],
                                     start=True, stop=True)
                    part = smp.tile([1, B], BF16, tag="part", name=nm("pa", t))
                    nc.vector.tensor_copy(part[:], pp0[:])
                    cin1 = dramp.tile([129, B], BF16, tag="ci1", name=nm("ci1", t))
                    cout1 = dramp.tile([NKH * 129, B], BF16, tag="co1",
                                       name=nm("co1", t), addr_space="Shared")
                    nc.sync.dma_start(out=cin1[0:128, :], in_=h1sl[:])
                    nc.sync.dma_start(out=cin1[128:129, :], in_=part[:])
                    nc.gpsimd.collective_compute(
                        "AllGather", ALU.bypass, ins=[cin1[:]], outs=[cout1[:]],
                        replica_groups=RG)
                    h1f = hfp.tile([128, NKH, B], BF16, tag="h1f",
                                   name=nm("hfh1f", t))
                    nc.sync.dma_start(
                        out=h1f[:],
                        in_=cout1[:].rearrange("(k q) b -> q k b", k=NKH)[0:128])
                    parts = smp.tile([NKH, B], BF16, tag="parts", name=nm("pts", t))
                    nc.sync.dma_start(
                        out=parts[:],
                        in_=cout1[:].rearrange("(k q) b -> q k b", k=NKH)[128:129])
                    pp = gpsum.tile([1, B], F32, tag="g", name=nm("pp", t))
                    nc.tensor.matmul(pp[:], ones8[:], parts[:],
                                     start=True, stop=True)
                    ps = smp.tile([1, B], F32, tag="pred", name=nm("ps", t))
                    nc.vector.tensor_scalar_add(ps[:], pp[:], b_proj)
                    nc.sync.dma_start(out=out_d[t:t + 1, :], in_=ps[:])
                    if t < T - 1:
                        xn = xt[(t + 1) % 2]
                        nc.sync.dma_start(out=xn[1:33, :], in_=knT_d[t + 1])
                        if tf_mask[t]:
                            nc.sync.dma_start(out=xn[0:1, :], in_=yT_d[t:t + 1, :])
                        else:
                            nc.vector.tensor_copy(xn[0:1, :], ps[:])

    nc.finalize()
    return nc


def assemble(res, T):
    out = np.zeros((B, T, 1), np.float32)
    out[:, :, 0] = res.results[0]["out"].T
    return out


def kernel(**inputs):
    from concourse.bass_utils import run_bass_kernel_spmd
    T = T_FULL
    shared, per_core, tf_mask, b_proj = prep_host(inputs, T)
    nc = build_module(T, tf_mask, b_proj)
    in_maps = []
    for c in range(N_CORES):
        m = dict(shared)
        m.update(per_core[c])
        in_maps.append(m)
    res = run_bass_kernel_spmd(nc, in_maps, list(range(N_CORES)))
    return assemble(res, T)
